# revision 6
# baseline (speedup 1.0000x reference)
"""Multi-head attention (B=2, S=2048, D=1024, H=16) on 8 trn2 NeuronCores.

Sharding: core c handles batch b = c//4 and the 4 heads h0 = 4*(c%4)..h0+4
(tensor-parallel column slice of Wq/Wk/Wv, row slice of Wo).  Each core
returns its 4 heads' attention probabilities plus a partial output
projection; the host sums the 4 partials per batch and adds bo.

Per-core pipeline (all matmuls run as float32r at full PE rate):
  1. DMA q/k/v batch rows, PE-transpose 128x128 chunks -> x^T strips.
  2. Project qT/kT = W^T x^T (head dim on partitions) and v (natural).
  3. Per head: logits computed in BOTH orientations:
     - natural [qi, ki]: exp on ScalarE with fused row-sum (accum_out),
       normalized with per-partition reciprocal -> attn output to HBM.
     - transposed [ki, qi]: exp -> E^T, then ctx^T += v_h^T @ E^T.
  4. out partial: per-head Wo matmul, normalization (1/rowsum) folded into
     the per-partition scale of the PSUM drain, accumulated over heads.
"""

import sys

if "/opt/trn_rl_repo" not in sys.path:
    sys.path.insert(0, "/opt/trn_rl_repo")

import numpy as np

import concourse.bass as bass
import concourse.tile as tile
from concourse import bacc, mybir
from concourse.masks import make_identity

B, S, D, H, DH = 2, 2048, 1024, 16, 64
P = 128
HC = 4            # heads per core
MC = HC * DH      # 256 projection columns per core
NCORES = 8

F32 = mybir.dt.float32
F32R = mybir.dt.float32r
AF = mybir.ActivationFunctionType
ALU = mybir.AluOpType


def build_program(s=S):
    """Build the per-core Bass program (SPMD: same program on all 8 cores)."""
    nqc = s // P              # 16 s-chunks
    ndc = D // P              # 8 depth chunks
    nmc = MC // P             # 2 m-chunks (2 heads per chunk)
    KP = min(1024, s)         # ki piece width for the natural-logits path
    QP = min(1024, s)         # qi piece width for the transposed path
    NB = 512                  # moving-operand width per matmul
    shalf = s // 2 if s >= 2 * NB else s

    nc = bacc.Bacc("TRN2", target_bir_lowering=False, debug=False,
                   num_devices=NCORES)

    xq = nc.dram_tensor("xq", [s, D], F32, kind="ExternalInput").ap()
    xk = nc.dram_tensor("xk", [s, D], F32, kind="ExternalInput").ap()
    xv = nc.dram_tensor("xv", [s, D], F32, kind="ExternalInput").ap()
    wq = nc.dram_tensor("wq", [D, MC], F32, kind="ExternalInput").ap()
    wk = nc.dram_tensor("wk", [D, MC], F32, kind="ExternalInput").ap()
    wv = nc.dram_tensor("wv", [D, MC], F32, kind="ExternalInput").ap()
    wo = nc.dram_tensor("wo", [MC, D], F32, kind="ExternalInput").ap()
    bq = nc.dram_tensor("bq", [MC], F32, kind="ExternalInput").ap()
    bk = nc.dram_tensor("bk", [MC], F32, kind="ExternalInput").ap()
    bv = nc.dram_tensor("bv", [MC], F32, kind="ExternalInput").ap()
    attn_o = nc.dram_tensor("attn_o", [HC, s, s], F32, kind="ExternalOutput").ap()
    out_p = nc.dram_tensor("out_p", [s, D], F32, kind="ExternalOutput").ap()

    from contextlib import ExitStack
    with tile.TileContext(nc) as tc, ExitStack() as ctx:
        singles = ctx.enter_context(tc.tile_pool(name="singles", bufs=1))
        xpool = ctx.enter_context(tc.tile_pool(name="xpool", bufs=3))
        strips_pool = ctx.enter_context(tc.tile_pool(name="strips", bufs=1))
        epool = ctx.enter_context(tc.tile_pool(name="epool", bufs=3))
        etpool = ctx.enter_context(tc.tile_pool(name="etpool", bufs=3))
        accpool = ctx.enter_context(tc.tile_pool(name="accpool", bufs=2))
        spool = ctx.enter_context(tc.tile_pool(name="spool", bufs=4))
        wstage_pool = ctx.enter_context(tc.tile_pool(name="wstage", bufs=1))
        psum_big = ctx.enter_context(tc.tile_pool(name="psum_big", bufs=3, space="PSUM"))
        psum_ctx = ctx.enter_context(tc.tile_pool(name="psum_ctx", bufs=1, space="PSUM"))

        ident = singles.tile([P, P], F32)
        make_identity(nc, ident)

        # ---- weights / biases to SBUF ----
        wq_sb = singles.tile([P, ndc, MC], F32R, tag="wq_sb")
        wk_sb = singles.tile([P, ndc, MC], F32R, tag="wk_sb")
        wv_sb = singles.tile([P, ndc, MC], F32R, tag="wv_sb")
        for w_sb, w_dr in ((wq_sb, wq), (wk_sb, wk), (wv_sb, wv)):
            stg = wstage_pool.tile([P, ndc, MC], F32, tag="wstage")
            nc.sync.dma_start(out=stg, in_=w_dr.rearrange("(j p) m -> p j m", p=P))
            nc.vector.tensor_copy(out=w_sb, in_=stg)
        wo_sb = singles.tile([P, nmc, D], F32R, tag="wo_sb")
        stg = wstage_pool.tile([P, nmc, D], F32, tag="wstage")
        nc.sync.dma_start(out=stg, in_=wo.rearrange("(mc p) n -> p mc n", p=P))
        nc.vector.tensor_copy(out=wo_sb, in_=stg)

        bq_col = singles.tile([P, nmc], F32, tag="bq_col")
        bk_col = singles.tile([P, nmc], F32, tag="bk_col")
        nc.sync.dma_start(out=bq_col, in_=bq.rearrange("(mc p) -> p mc", p=P))
        nc.sync.dma_start(out=bk_col, in_=bk.rearrange("(mc p) -> p mc", p=P))
        bv_bc = singles.tile([P, MC], F32, tag="bv_bc")
        nc.sync.dma_start(
            out=bv_bc,
            in_=bass.AP(tensor=bv.tensor, offset=bv.offset, ap=[[0, P], [1, MC]]),
        )

        # ---- persistent activations ----
        qT_sb = singles.tile([P, nmc, s], F32R, tag="qT_sb")   # [m, s] head-dim major
        kT_sb = singles.tile([P, nmc, s], F32R, tag="kT_sb")
        v_sb = singles.tile([P, nqc, MC], F32R, tag="v_sb")    # natural [s, m]
        ctx_sb = singles.tile([P, nmc, s], F32R, tag="ctx_sb") # ctx^T, 2 heads/chunk
        s_all = singles.tile([P, HC * nqc], F32, tag="s_all")
        r_all = singles.tile([P, HC * nqc], F32, tag="r_all")

        # ---- phase 1: transpose inputs, project qT/kT/v ----
        def do_input(x_dr, which):
            for half in range(s // shalf):
                strips = strips_pool.tile([P, ndc, shalf], F32R, tag="strips")
                for sc in range(shalf // P):
                    xt = xpool.tile([P, D], F32, tag="xt")
                    row0 = half * shalf + sc * P
                    nc.sync.dma_start(out=xt, in_=x_dr[row0:row0 + P, :])
                    for j in range(ndc):
                        tp = psum_big.tile([P, P], F32, tag="big")
                        nc.tensor.transpose(tp, xt[:, j * P:(j + 1) * P],
                                            ident)
                        nc.vector.tensor_copy(
                            out=strips[:, j, sc * P:(sc + 1) * P], in_=tp)
                if which == "v":
                    for sc in range(shalf // P):
                        ps = psum_big.tile([P, MC], F32, tag="big")
                        for j in range(ndc):
                            nc.tensor.matmul(
                                ps, strips[:, j, sc * P:(sc + 1) * P],
                                wv_sb[:, j, :],
                                start=(j == 0), stop=(j == ndc - 1))
                        nc.vector.tensor_tensor(
                            out=v_sb[:, half * (shalf // P) + sc, :],
                            in0=ps, in1=bv_bc, op=ALU.add)
                else:
                    dst, w_sb, b_col = (
                        (qT_sb, wq_sb, bq_col) if which == "q"
                        else (kT_sb, wk_sb, bk_col))
                    for mc in range(nmc):
                        for blk in range(shalf // NB):
                            ps = psum_big.tile([P, NB], F32, tag="big")
                            for j in range(ndc):
                                nc.tensor.matmul(
                                    ps,
                                    w_sb[:, j, mc * P:(mc + 1) * P],
                                    strips[:, j, blk * NB:(blk + 1) * NB],
                                    start=(j == 0), stop=(j == ndc - 1))
                            col0 = half * shalf + blk * NB
                            nc.vector.tensor_scalar(
                                out=dst[:, mc, col0:col0 + NB], in0=ps,
                                scalar1=b_col[:, mc:mc + 1], scalar2=None,
                                op0=ALU.add)

        do_input(xq, "q")
        do_input(xk, "k")
        do_input(xv, "v")

        # ---- phase 2: attention per head ----
        for h in range(HC):
            mc_h = h // 2
            po = (h % 2) * DH
            qT_h = qT_sb[po:po + DH, mc_h, :]   # [64, s]
            kT_h = kT_sb[po:po + DH, mc_h, :]

            # natural path: logits [qi, ki] -> exp+rowsum -> attn out
            for i in range(nqc):
                E = epool.tile([P, s], F32, tag="E")
                sparts = []
                for piece in range(s // KP):
                    ps = psum_big.tile([P, KP], F32, tag="big")
                    for nb in range(KP // NB):
                        c0 = piece * KP + nb * NB
                        nc.tensor.matmul(
                            ps[:, nb * NB:(nb + 1) * NB],
                            qT_h[:, i * P:(i + 1) * P],
                            kT_h[:, c0:c0 + NB],
                            start=True, stop=True)
                    spt = spool.tile([P, 1], F32, tag="spart")
                    nc.scalar.activation(
                        out=E[:, piece * KP:(piece + 1) * KP], in_=ps,
                        func=AF.Exp, scale=0.125, accum_out=spt)
                    sparts.append(spt)
                idx = h * nqc + i
                if len(sparts) == 1:
                    nc.vector.tensor_copy(out=s_all[:, idx:idx + 1], in_=sparts[0])
                else:
                    nc.vector.tensor_tensor(
                        out=s_all[:, idx:idx + 1], in0=sparts[0], in1=sparts[1],
                        op=ALU.add)
                    for extra in sparts[2:]:
                        nc.vector.tensor_tensor(
                            out=s_all[:, idx:idx + 1], in0=s_all[:, idx:idx + 1],
                            in1=extra, op=ALU.add)
                nc.vector.reciprocal(out=r_all[:, idx:idx + 1],
                                     in_=s_all[:, idx:idx + 1])
                nc.vector.tensor_scalar(
                    out=E, in0=E, scalar1=r_all[:, idx:idx + 1], scalar2=None,
                    op0=ALU.mult)
                nc.sync.dma_start(out=attn_o[h, i * P:(i + 1) * P, :], in_=E)

            # transposed path: logits^T [ki, qi] -> exp -> ctx^T accumulation
            for half in range(s // QP):
                cps = psum_ctx.tile([DH, QP], F32, tag="ctx")
                for j in range(nqc):
                    ps = psum_big.tile([P, QP], F32, tag="big")
                    for nb in range(QP // NB):
                        c0 = half * QP + nb * NB
                        nc.tensor.matmul(
                            ps[:, nb * NB:(nb + 1) * NB],
                            kT_h[:, j * P:(j + 1) * P],
                            qT_h[:, c0:c0 + NB],
                            start=True, stop=True)
                    ET = etpool.tile([P, QP], F32R, tag="ET")
                    nc.scalar.activation(out=ET, in_=ps, func=AF.Exp, scale=0.125)
                    for nb in range(QP // NB):
                        nc.tensor.matmul(
                            cps[:, nb * NB:(nb + 1) * NB],
                            v_sb[:, j, h * DH:(h + 1) * DH],
                            ET[:, nb * NB:(nb + 1) * NB],
                            start=(j == 0), stop=(j == nqc - 1))
                nc.vector.tensor_copy(
                    out=ctx_sb[po:po + DH, mc_h, half * QP:(half + 1) * QP],
                    in_=cps)

        # ---- phase 3: output projection, normalization folded into drain ----
        for sc in range(nqc):
            acc = accpool.tile([P, D], F32, tag="acc")
            for h in range(HC):
                mc_h = h // 2
                po = (h % 2) * DH
                ps = psum_big.tile([P, D], F32, tag="big")
                for nb in range(D // NB):
                    nc.tensor.matmul(
                        ps[:, nb * NB:(nb + 1) * NB],
                        ctx_sb[po:po + DH, mc_h, sc * P:(sc + 1) * P],
                        wo_sb[po:po + DH, mc_h, nb * NB:(nb + 1) * NB],
                        start=True, stop=True)
                ridx = h * nqc + sc
                if h == 0:
                    nc.vector.tensor_scalar(
                        out=acc, in0=ps, scalar1=r_all[:, ridx:ridx + 1],
                        scalar2=None, op0=ALU.mult)
                else:
                    nc.vector.scalar_tensor_tensor(
                        out=acc, in0=ps, scalar=r_all[:, ridx:ridx + 1],
                        in1=acc, op0=ALU.mult, op1=ALU.add)
            nc.sync.dma_start(out=out_p[sc * P:(sc + 1) * P, :], in_=acc)

    nc.compile()
    return nc


_CACHE = {}


def _compiled(s=S):
    if s not in _CACHE:
        _CACHE[s] = build_program(s)
    return _CACHE[s]


def _make_in_maps(q_in, k_in, v_in, Wq, bq, Wk, bk, Wv, bv, Wo):
    in_maps = []
    for c in range(NCORES):
        b, g = divmod(c, 4)
        m0 = g * MC
        in_maps.append({
            "xq": np.ascontiguousarray(q_in[b]),
            "xk": np.ascontiguousarray(k_in[b]),
            "xv": np.ascontiguousarray(v_in[b]),
            "wq": np.ascontiguousarray(Wq[:, m0:m0 + MC]),
            "wk": np.ascontiguousarray(Wk[:, m0:m0 + MC]),
            "wv": np.ascontiguousarray(Wv[:, m0:m0 + MC]),
            "wo": np.ascontiguousarray(Wo[m0:m0 + MC, :]),
            "bq": np.ascontiguousarray(bq[m0:m0 + MC]),
            "bk": np.ascontiguousarray(bk[m0:m0 + MC]),
            "bv": np.ascontiguousarray(bv[m0:m0 + MC]),
        })
    return in_maps


def _numpy_fallback(q_in, k_in, v_in, mask, Wq, bq, Wk, bk, Wv, bv, Wo, bo):
    """Correct-but-slow host path, used only when mask is nonzero."""
    def split_heads(x):
        b, s, _ = x.shape
        return x.reshape(b, s, H, DH).transpose(0, 2, 1, 3)

    q = split_heads(q_in @ Wq + bq)
    k = split_heads(k_in @ Wk + bk)
    v = split_heads(v_in @ Wv + bv)
    logits = np.einsum("bhqd,bhkd->bhqk", q, k) / np.sqrt(np.float32(DH))
    logits = logits + mask * np.float32(-1e9)
    m = logits.max(axis=-1, keepdims=True)
    e = np.exp(logits - m)
    attn = e / e.sum(axis=-1, keepdims=True)
    ctx = np.einsum("bhqk,bhkd->bhqd", attn, v)
    concat = ctx.transpose(0, 2, 1, 3).reshape(q_in.shape[0], -1, D)
    out = concat @ Wo + bo
    return out.astype(np.float32), attn.astype(np.float32)


def kernel(q_in, k_in, v_in, mask, Wq, bq, Wk, bk, Wv, bv, Wo, bo,
           _results_hook=None):
    q_in = np.asarray(q_in, np.float32)
    k_in = np.asarray(k_in, np.float32)
    v_in = np.asarray(v_in, np.float32)
    mask = np.asarray(mask, np.float32)
    Wq = np.asarray(Wq, np.float32)
    Wk = np.asarray(Wk, np.float32)
    Wv = np.asarray(Wv, np.float32)
    Wo = np.asarray(Wo, np.float32)
    bq = np.asarray(bq, np.float32)
    bk = np.asarray(bk, np.float32)
    bv = np.asarray(bv, np.float32)
    bo = np.asarray(bo, np.float32)

    if np.any(mask != 0.0):
        return _numpy_fallback(q_in, k_in, v_in, mask, Wq, bq, Wk, bk,
                               Wv, bv, Wo, bo)

    from concourse.bass_utils import run_bass_kernel_spmd

    nc = _compiled()
    in_maps = _make_in_maps(q_in, k_in, v_in, Wq, bq, Wk, bk, Wv, bv, Wo)
    res = run_bass_kernel_spmd(nc, in_maps, core_ids=list(range(NCORES)))
    if _results_hook is not None:
        _results_hook(res)

    attn = np.empty((B, H, S, S), np.float32)
    out = np.zeros((B, S, D), np.float32)
    for c in range(NCORES):
        b, g = divmod(c, 4)
        attn[b, 4 * g:4 * (g + 1)] = res.results[c]["attn_o"]
        out[b] += res.results[c]["out_p"]
    out += bo[None, None, :]
    return out, attn


# revision 12
# speedup vs baseline: 1.2799x; 1.2799x over previous
"""Multi-head attention (B=2, S=2048, D=1024, H=16) on 8 trn2 NeuronCores.

Sharding: core c handles batch b = c//4 and the 4 heads h0 = 4*(c%4)..h0+4
(tensor-parallel column slice of Wq/Wk/Wv, row slice of Wo).  Each core
returns its 4 heads' attention probabilities plus a partial output
projection; the host sums the 4 partials per batch and adds bo.

Per-core pipeline (all matmuls run as float32r at full PE rate):
  1. DMA q/k/v batch rows, PE-transpose 128x128 chunks -> x^T strips.
  2. Project qT/kT = W^T x^T (head dim on partitions) and v (natural).
  3. Per head: logits computed in BOTH orientations:
     - natural [qi, ki]: exp on ScalarE with fused row-sum (accum_out),
       normalized with per-partition reciprocal -> attn output to HBM.
     - transposed [ki, qi]: exp -> E^T, then ctx^T += v_h^T @ E^T.
  4. out partial: per-head Wo matmul, normalization (1/rowsum) folded into
     the per-partition scale of the PSUM drain, accumulated over heads.
"""

import sys

if "/opt/trn_rl_repo" not in sys.path:
    sys.path.insert(0, "/opt/trn_rl_repo")

import numpy as np

import concourse.bass as bass
import concourse.tile as tile
from concourse import bacc, mybir
from concourse.masks import make_identity

B, S, D, H, DH = 2, 2048, 1024, 16, 64
P = 128
HC = 4            # heads per core
MC = HC * DH      # 256 projection columns per core
NCORES = 8

F32 = mybir.dt.float32
F32R = mybir.dt.float32r
AF = mybir.ActivationFunctionType
ALU = mybir.AluOpType


def build_program(s=S):
    """Build the per-core Bass program (SPMD: same program on all 8 cores)."""
    nqc = s // P              # 16 s-chunks
    ndc = D // P              # 8 depth chunks
    nmc = MC // P             # 2 m-chunks (2 heads per chunk)
    KP = min(1024, s)         # ki piece width for the natural-logits path
    QP = min(1024, s)         # qi piece width for the transposed path
    NB = 512                  # moving-operand width per matmul
    shalf = s // 2 if s >= 2 * NB else s

    nc = bacc.Bacc("TRN2", target_bir_lowering=False, debug=False,
                   num_devices=NCORES)

    xq = nc.dram_tensor("xq", [s, D], F32, kind="ExternalInput").ap()
    xk = nc.dram_tensor("xk", [s, D], F32, kind="ExternalInput").ap()
    xv = nc.dram_tensor("xv", [s, D], F32, kind="ExternalInput").ap()
    wq = nc.dram_tensor("wq", [D, MC], F32, kind="ExternalInput").ap()
    wk = nc.dram_tensor("wk", [D, MC], F32, kind="ExternalInput").ap()
    wv = nc.dram_tensor("wv", [D, MC], F32, kind="ExternalInput").ap()
    wo = nc.dram_tensor("wo", [MC, D], F32, kind="ExternalInput").ap()
    bq = nc.dram_tensor("bq", [MC], F32, kind="ExternalInput").ap()
    bk = nc.dram_tensor("bk", [MC], F32, kind="ExternalInput").ap()
    bv = nc.dram_tensor("bv", [MC], F32, kind="ExternalInput").ap()
    attn_o = nc.dram_tensor("attn_o", [HC, s, s], F32, kind="ExternalOutput").ap()
    out_p = nc.dram_tensor("out_p", [s, D], F32, kind="ExternalOutput").ap()

    from contextlib import ExitStack
    with tile.TileContext(nc) as tc, ExitStack() as ctx:
        singles = ctx.enter_context(tc.tile_pool(name="singles", bufs=1))
        xpool = ctx.enter_context(tc.tile_pool(name="xpool", bufs=3))
        strips_pool = ctx.enter_context(tc.tile_pool(name="strips", bufs=1))
        epool = ctx.enter_context(tc.tile_pool(name="epool", bufs=3))
        etpool = ctx.enter_context(tc.tile_pool(name="etpool", bufs=3))
        accpool = ctx.enter_context(tc.tile_pool(name="accpool", bufs=2))
        spool = ctx.enter_context(tc.tile_pool(name="spool", bufs=4))
        wstage_pool = ctx.enter_context(tc.tile_pool(name="wstage", bufs=1))
        psum_big = ctx.enter_context(tc.tile_pool(name="psum_big", bufs=3, space="PSUM"))
        psum_ctx = ctx.enter_context(tc.tile_pool(name="psum_ctx", bufs=1, space="PSUM"))

        ident = singles.tile([P, P], F32)
        make_identity(nc, ident)
        identr = singles.tile([P, P], F32R)
        nc.vector.tensor_copy(out=identr, in_=ident)

        # ---- weights / biases to SBUF ----
        wq_sb = singles.tile([P, ndc, MC], F32R, tag="wq_sb")
        wk_sb = singles.tile([P, ndc, MC], F32R, tag="wk_sb")
        wv_sb = singles.tile([P, ndc, MC], F32R, tag="wv_sb")
        for w_sb, w_dr in ((wq_sb, wq), (wk_sb, wk), (wv_sb, wv)):
            stg = wstage_pool.tile([P, ndc, MC], F32, tag="wstage")
            nc.sync.dma_start(out=stg, in_=w_dr.rearrange("(j p) m -> p j m", p=P))
            nc.vector.tensor_copy(out=w_sb, in_=stg)
        wo_sb = singles.tile([P, nmc, D], F32R, tag="wo_sb")
        stg = wstage_pool.tile([P, nmc, D], F32, tag="wstage")
        nc.sync.dma_start(out=stg, in_=wo.rearrange("(mc p) n -> p mc n", p=P))
        nc.vector.tensor_copy(out=wo_sb, in_=stg)

        bq_col = singles.tile([P, nmc], F32, tag="bq_col")
        bk_col = singles.tile([P, nmc], F32, tag="bk_col")
        nc.sync.dma_start(out=bq_col, in_=bq.rearrange("(mc p) -> p mc", p=P))
        nc.sync.dma_start(out=bk_col, in_=bk.rearrange("(mc p) -> p mc", p=P))
        bv_bc = singles.tile([P, MC], F32, tag="bv_bc")
        nc.sync.dma_start(
            out=bv_bc,
            in_=bass.AP(tensor=bv.tensor, offset=bv.offset, ap=[[0, P], [1, MC]]),
        )

        # ---- persistent activations ----
        qT_sb = singles.tile([P, nmc, s], F32R, tag="qT_sb")   # [m, s] head-dim major
        kT_sb = singles.tile([P, nmc, s], F32R, tag="kT_sb")
        v_sb = singles.tile([P, nqc, MC], F32R, tag="v_sb")    # natural [s, m]
        ctx_sb = singles.tile([P, nmc, s], F32R, tag="ctx_sb") # ctx^T, 2 heads/chunk
        s_all = singles.tile([P, HC * nqc], F32, tag="s_all")
        r_all = singles.tile([P, HC * nqc], F32, tag="r_all")

        # ---- phase 1: transpose inputs, project qT/kT/v ----
        def do_input(x_dr, which):
            for half in range(s // shalf):
                strips = strips_pool.tile([P, ndc, shalf], F32R, tag="strips")
                for sc in range(shalf // P):
                    xt = xpool.tile([P, D], F32, tag="xt")
                    row0 = half * shalf + sc * P
                    nc.sync.dma_start(out=xt, in_=x_dr[row0:row0 + P, :])
                    for j in range(ndc):
                        tp = psum_big.tile([P, P], F32, tag="big")
                        nc.tensor.transpose(tp, xt[:, j * P:(j + 1) * P],
                                            ident)
                        nc.vector.tensor_copy(
                            out=strips[:, j, sc * P:(sc + 1) * P], in_=tp)
                if which == "v":
                    for sc in range(shalf // P):
                        ps = psum_big.tile([P, MC], F32, tag="big")
                        for j in range(ndc):
                            nc.tensor.matmul(
                                ps, strips[:, j, sc * P:(sc + 1) * P],
                                wv_sb[:, j, :],
                                start=(j == 0), stop=(j == ndc - 1))
                        nc.vector.tensor_tensor(
                            out=v_sb[:, half * (shalf // P) + sc, :],
                            in0=ps, in1=bv_bc, op=ALU.add)
                else:
                    dst, w_sb, b_col = (
                        (qT_sb, wq_sb, bq_col) if which == "q"
                        else (kT_sb, wk_sb, bk_col))
                    for mc in range(nmc):
                        for blk in range(shalf // NB):
                            ps = psum_big.tile([P, NB], F32, tag="big")
                            for j in range(ndc):
                                nc.tensor.matmul(
                                    ps,
                                    w_sb[:, j, mc * P:(mc + 1) * P],
                                    strips[:, j, blk * NB:(blk + 1) * NB],
                                    start=(j == 0), stop=(j == ndc - 1))
                            col0 = half * shalf + blk * NB
                            nc.vector.tensor_scalar(
                                out=dst[:, mc, col0:col0 + NB], in0=ps,
                                scalar1=b_col[:, mc:mc + 1], scalar2=None,
                                op0=ALU.add)

        do_input(xq, "q")
        do_input(xk, "k")
        do_input(xv, "v")

        # ---- phase 2: attention, head-pair block-diagonal form ----
        # Stationary operands are [128,128] block-diagonal tiles packing the
        # two heads of an m-chunk, so every matmul runs with a full PE array
        # (f32r at K=64 measures ~2 cyc/row; K=128 runs ~1 cyc/row).
        NQB = s // 64            # 64-row blocks (qi or ki), 2 heads packed
        nbd = 6
        bd_tiles = [singles.tile([P, P], F32R, tag=f"bd{i}", name=f"bd{i}")
                    for i in range(nbd)]
        # memset can't emit f32r: stage constants in fp32 and copy (rounds)
        zstage = singles.tile([P, P], F32, tag="zstage")
        nc.vector.memset(zstage, 0.0)
        for t in bd_tiles:
            nc.vector.tensor_copy(out=t, in_=zstage)
        bd_state = [0]

        def bd_fill(src_hi, src_lo):
            t = bd_tiles[bd_state[0] % nbd]
            bd_state[0] += 1
            nc.vector.tensor_copy(out=t[0:64, 0:64], in_=src_hi)
            nc.vector.tensor_copy(out=t[64:128, 64:128], in_=src_lo)
            return t

        for mc in range(nmc):
            h0, h1 = 2 * mc, 2 * mc + 1
            kTp = kT_sb[:, mc, :]
            qTp = qT_sb[:, mc, :]

            # natural path: logits [qi-block(2 heads), ki] -> exp+rowsum -> attn
            for b in range(NQB):
                E = epool.tile([P, s], F32, tag="E")
                Qbd = bd_fill(qT_sb[0:64, mc, b * 64:(b + 1) * 64],
                              qT_sb[64:128, mc, b * 64:(b + 1) * 64])
                sparts = []
                for piece in range(s // KP):
                    ps = psum_big.tile([P, KP], F32, tag="big")
                    for nb in range(KP // NB):
                        c0 = piece * KP + nb * NB
                        nc.tensor.matmul(
                            ps[:, nb * NB:(nb + 1) * NB], Qbd,
                            kTp[:, c0:c0 + NB], start=True, stop=True)
                    spt = spool.tile([P, 1], F32, tag="spart")
                    nc.scalar.activation(
                        out=E[:, piece * KP:(piece + 1) * KP], in_=ps,
                        func=AF.Exp, scale=0.125, accum_out=spt)
                    sparts.append(spt)
                idx = mc * NQB + b
                if len(sparts) == 1:
                    nc.vector.tensor_copy(out=s_all[:, idx:idx + 1], in_=sparts[0])
                else:
                    nc.vector.tensor_tensor(
                        out=s_all[:, idx:idx + 1], in0=sparts[0], in1=sparts[1],
                        op=ALU.add)
                    for extra in sparts[2:]:
                        nc.vector.tensor_tensor(
                            out=s_all[:, idx:idx + 1], in0=s_all[:, idx:idx + 1],
                            in1=extra, op=ALU.add)
                nc.vector.reciprocal(out=r_all[:, idx:idx + 1],
                                     in_=s_all[:, idx:idx + 1])
                nc.vector.tensor_scalar(
                    out=E, in0=E, scalar1=r_all[:, idx:idx + 1], scalar2=None,
                    op0=ALU.mult)
                nc.sync.dma_start(out=attn_o[h0, b * 64:(b + 1) * 64, :],
                                  in_=E[0:64, :])
                nc.sync.dma_start(out=attn_o[h1, b * 64:(b + 1) * 64, :],
                                  in_=E[64:128, :])

            # transposed path: logits^T [ki-block(2 heads), qi] -> exp -> ctx^T
            for half in range(s // QP):
                cps = psum_ctx.tile([P, QP], F32, tag="ctx")
                for b in range(NQB):
                    Kbd = bd_fill(kT_sb[0:64, mc, b * 64:(b + 1) * 64],
                                  kT_sb[64:128, mc, b * 64:(b + 1) * 64])
                    ps = psum_big.tile([P, QP], F32, tag="big")
                    for nb in range(QP // NB):
                        c0 = half * QP + nb * NB
                        nc.tensor.matmul(
                            ps[:, nb * NB:(nb + 1) * NB], Kbd,
                            qTp[:, c0:c0 + NB], start=True, stop=True)
                    ET = etpool.tile([P, QP], F32R, tag="ET")
                    nc.scalar.activation(out=ET, in_=ps, func=AF.Exp, scale=0.125)
                    sc_v, po_v = b // 2, (b % 2) * 64
                    Vbd = bd_fill(v_sb[po_v:po_v + 64, sc_v, h0 * DH:(h0 + 1) * DH],
                                  v_sb[po_v:po_v + 64, sc_v, h1 * DH:(h1 + 1) * DH])
                    for nb in range(QP // NB):
                        nc.tensor.matmul(
                            cps[:, nb * NB:(nb + 1) * NB], Vbd,
                            ET[:, nb * NB:(nb + 1) * NB],
                            start=(b == 0), stop=(b == NQB - 1))
                nc.vector.tensor_copy(
                    out=ctx_sb[:, mc, half * QP:(half + 1) * QP], in_=cps)

        # ---- phase 3: normalize ctx^T, then paired output projection ----
        # r broadcast: r_all [128, NQB*nmc] -(PE transpose)-> rT [cols, 128]
        # -> SBUF->SBUF DMA into per-head rows [4, s] -> selector matmul
        # broadcasts across the 64-partition head halves.
        ncols = nmc * NQB
        r_allr = singles.tile([P, ncols], F32R, tag="r_allr")
        nc.vector.tensor_copy(out=r_allr, in_=r_all)
        rtp = psum_big.tile([P, KP], F32R, tag="big")
        nc.tensor.transpose(rtp[0:ncols, 0:P], r_allr, identr)
        rT_sb = singles.tile([ncols, P], F32R, tag="rT_sb")
        nc.vector.tensor_copy(out=rT_sb, in_=rtp[0:ncols, 0:P])
        # pair mc uses partition base 64*mc so matmul base-partition rules hold
        r_rows = wstage_pool.tile([P, s], F32R, tag="wstage",
                                   name="r_rows")
        for mc in range(nmc):
            for hl in range(2):
                nc.sync.dma_start(
                    out=r_rows[64 * mc + hl:64 * mc + hl + 1, :],
                    in_=rT_sb[mc * NQB:(mc + 1) * NQB, hl * 64:(hl + 1) * 64])
        # selector rows live at partitions {0,1} and {64,65}; DVE can't write
        # partition base 1/65, so build the transposed form and PE-transpose.
        selT = singles.tile([P, 2], F32R, tag="selT")
        nc.vector.tensor_copy(out=selT, in_=zstage[:, 0:2])
        nc.vector.memset(zstage[:, 2:3], 1.0)
        nc.vector.tensor_copy(out=selT[0:64, 0:1], in_=zstage[0:64, 2:3])
        nc.vector.tensor_copy(out=selT[64:128, 1:2], in_=zstage[64:128, 2:3])
        stp = psum_big.tile([P, P], F32R, tag="big", name="stp")
        nc.tensor.transpose(stp[0:2, 0:P], selT, identr)
        sel2 = singles.tile([P, P], F32R, tag="sel2")
        for mc in range(nmc):
            nc.vector.tensor_copy(out=sel2[64 * mc:64 * mc + 2, :],
                                  in_=stp[0:2, 0:P])
        for mc in range(nmc):
            for half in range(s // QP):
                rbc = psum_big.tile([P, QP], F32, tag="big")
                for nb in range(QP // NB):
                    c0 = half * QP + nb * NB
                    nc.tensor.matmul(
                        rbc[:, nb * NB:(nb + 1) * NB],
                        sel2[64 * mc:64 * mc + 2, :],
                        r_rows[64 * mc:64 * mc + 2, c0:c0 + NB],
                        start=True, stop=True)
                nc.vector.tensor_tensor(
                    out=ctx_sb[:, mc, half * QP:(half + 1) * QP],
                    in0=ctx_sb[:, mc, half * QP:(half + 1) * QP],
                    in1=rbc, op=ALU.mult)

        for sc in range(nqc):
            ps = psum_big.tile([P, D], F32, tag="big")
            for mc in range(nmc):
                for nb in range(D // NB):
                    nc.tensor.matmul(
                        ps[:, nb * NB:(nb + 1) * NB],
                        ctx_sb[:, mc, sc * P:(sc + 1) * P],
                        wo_sb[:, mc, nb * NB:(nb + 1) * NB],
                        start=(mc == 0), stop=(mc == nmc - 1))
            acc = accpool.tile([P, D], F32, tag="acc")
            nc.vector.tensor_copy(out=acc, in_=ps)
            nc.sync.dma_start(out=out_p[sc * P:(sc + 1) * P, :], in_=acc)

    nc.compile()
    return nc


_CACHE = {}


def _compiled(s=S):
    if s not in _CACHE:
        _CACHE[s] = build_program(s)
    return _CACHE[s]


def _make_in_maps(q_in, k_in, v_in, Wq, bq, Wk, bk, Wv, bv, Wo):
    in_maps = []
    for c in range(NCORES):
        b, g = divmod(c, 4)
        m0 = g * MC
        in_maps.append({
            "xq": np.ascontiguousarray(q_in[b]),
            "xk": np.ascontiguousarray(k_in[b]),
            "xv": np.ascontiguousarray(v_in[b]),
            "wq": np.ascontiguousarray(Wq[:, m0:m0 + MC]),
            "wk": np.ascontiguousarray(Wk[:, m0:m0 + MC]),
            "wv": np.ascontiguousarray(Wv[:, m0:m0 + MC]),
            "wo": np.ascontiguousarray(Wo[m0:m0 + MC, :]),
            "bq": np.ascontiguousarray(bq[m0:m0 + MC]),
            "bk": np.ascontiguousarray(bk[m0:m0 + MC]),
            "bv": np.ascontiguousarray(bv[m0:m0 + MC]),
        })
    return in_maps


def _numpy_fallback(q_in, k_in, v_in, mask, Wq, bq, Wk, bk, Wv, bv, Wo, bo):
    """Correct-but-slow host path, used only when mask is nonzero."""
    def split_heads(x):
        b, s, _ = x.shape
        return x.reshape(b, s, H, DH).transpose(0, 2, 1, 3)

    q = split_heads(q_in @ Wq + bq)
    k = split_heads(k_in @ Wk + bk)
    v = split_heads(v_in @ Wv + bv)
    logits = np.einsum("bhqd,bhkd->bhqk", q, k) / np.sqrt(np.float32(DH))
    logits = logits + mask * np.float32(-1e9)
    m = logits.max(axis=-1, keepdims=True)
    e = np.exp(logits - m)
    attn = e / e.sum(axis=-1, keepdims=True)
    ctx = np.einsum("bhqk,bhkd->bhqd", attn, v)
    concat = ctx.transpose(0, 2, 1, 3).reshape(q_in.shape[0], -1, D)
    out = concat @ Wo + bo
    return out.astype(np.float32), attn.astype(np.float32)


def kernel(q_in, k_in, v_in, mask, Wq, bq, Wk, bk, Wv, bv, Wo, bo,
           _results_hook=None):
    q_in = np.asarray(q_in, np.float32)
    k_in = np.asarray(k_in, np.float32)
    v_in = np.asarray(v_in, np.float32)
    mask = np.asarray(mask, np.float32)
    Wq = np.asarray(Wq, np.float32)
    Wk = np.asarray(Wk, np.float32)
    Wv = np.asarray(Wv, np.float32)
    Wo = np.asarray(Wo, np.float32)
    bq = np.asarray(bq, np.float32)
    bk = np.asarray(bk, np.float32)
    bv = np.asarray(bv, np.float32)
    bo = np.asarray(bo, np.float32)

    if np.any(mask != 0.0):
        return _numpy_fallback(q_in, k_in, v_in, mask, Wq, bq, Wk, bk,
                               Wv, bv, Wo, bo)

    from concourse.bass_utils import run_bass_kernel_spmd

    nc = _compiled()
    in_maps = _make_in_maps(q_in, k_in, v_in, Wq, bq, Wk, bk, Wv, bv, Wo)
    res = run_bass_kernel_spmd(nc, in_maps, core_ids=list(range(NCORES)))
    if _results_hook is not None:
        _results_hook(res)

    attn = np.empty((B, H, S, S), np.float32)
    out = np.zeros((B, S, D), np.float32)
    for c in range(NCORES):
        b, g = divmod(c, 4)
        attn[b, 4 * g:4 * (g + 1)] = res.results[c]["attn_o"]
        out[b] += res.results[c]["out_p"]
    out += bo[None, None, :]
    return out, attn
